# revision 27
# baseline (speedup 1.0000x reference)
"""Trainium2 Bass kernel for CustomSNNLoss (nn_CustomSNNLoss_36429912604816).

Strategy (data-parallel over rows of the NxN similarity):
  - Host: normalize x, quantize to bf16, pre-transpose to xnT [128, N];
    build per-key one-hot class matrices in fp8 (exact 0/1 values).
  - Each of the 8 cores owns R = 768 query rows, split into asymmetric
    q-chunks A (512 cols, pair-batched 2KB-aligned sim slots) and B
    (256 cols, quad-batched 1KB sim slots) so exp/square run as 36 wide
    instructions with PSUM = 4 sim banks + 4 exclusive acc banks:
        sim      = xnT[:, ktile].T @ xnq          (bf16 matmul -> PSUM f32)
        Sb       = exp(2 * sim) batch-batched     (ScalarE, fp8 out)
        St       = Sb * Sb                        (DVE, fp8)
        acc_b   += onehot_combo[pair].T  @ Sb     (fp8 DoubleRow, 256-key
        acc_t   += onehot_target[pair].T @ St      contraction per matmul)
    fp8 quantization of the exp values is harmless: errors average out in
    the ~300-element segment sums and biases cancel in pos/(pos+neg)
    (host-emulated end-to-end rel err ~1e-6).
  - Host epilogue (O(N)): per-row pos/neg sums from the class sums, -log
    losses, validity masks, class-weighted means, final scalar. Diagonal
    subtraction uses host emulation of the device's bf16/fp8 rounding.
"""

import numpy as np

N, D = 6144, 128
P = 128                 # partitions / contraction tile
NCORES = 8
R = N // NCORES         # 768 query rows per core
KT = N // P             # 48 key tiles
NPAIR = KT // 2         # 24 key-tile pairs (DoubleRow contracts 2 tiles)
QC = 2                  # query chunks per core
QF = R // QC            # 384 free-dim per matmul
NT, NB = 20, 5          # target classes, batch keys
NCB = NT * NB           # 100 combined classes
# DoubleRow LDWEIGHTS requires the k-subtile stride %16==0 and 16-aligned
# slice offsets: pad the one-hot row to 144 cols; target at [0:20],
# combo at [32:132].
CW = 144
TOFF, BOFF = 0, 32
XCH = 6                 # xnt DMA chunks (8 k-tiles each)
WCH = 3                 # weight DMA chunks (8 pairs each)
MIN_T, MAX_T = 0.1, 1.0
TEMP_BATCH = 0.5
EPS = 1e-8

_compile_cache = {}
LAST_RESULT = None  # BassKernelResults from the most recent device run


def _build(scale_t: float, scale_b: float, square_mode: bool):
    from contextlib import ExitStack

    import concourse.bacc as bacc
    import concourse.mybir as mybir
    import concourse.tile as tile

    f32 = mybir.dt.float32
    bf16 = mybir.dt.bfloat16
    fp8 = mybir.dt.float8e4
    EXP = mybir.ActivationFunctionType.Exp
    DR = mybir.MatmulPerfMode.DoubleRow

    nc = bacc.Bacc("TRN2", target_bir_lowering=False, debug=False,
                   enable_asserts=False)

    KC = KT // XCH * P      # key columns per xnt chunk
    xnt = nc.dram_tensor("xnt", [XCH, P, KC], bf16, kind="ExternalInput").ap()
    xnq = nc.dram_tensor("xnq", [P, R], bf16, kind="ExternalInput").ap()
    PW = NPAIR // WCH       # pairs per weight chunk (8)
    wcb = nc.dram_tensor("wcb", [WCH, P, PW, 2, CW], fp8,
                         kind="ExternalInput").ap()
    out_b = nc.dram_tensor("out_b", [NCB, R], f32, kind="ExternalOutput").ap()
    out_t = nc.dram_tensor("out_t", [NT, R], f32, kind="ExternalOutput").ap()

    with tile.TileContext(nc) as tc, ExitStack() as ctx:
        const = ctx.enter_context(tc.tile_pool(name="const", bufs=1))
        work = ctx.enter_context(tc.tile_pool(name="work", bufs=4))
        psim = ctx.enter_context(tc.tile_pool(name="psim", bufs=2, space="PSUM"))
        pacc = ctx.enter_context(tc.tile_pool(name="pacc", bufs=1, space="PSUM"))

        xnt_sb = const.tile([P, N], bf16, name="xnt_sb")
        xnq_sb = const.tile([P, R], bf16, name="xnq_sb")
        w_sb = const.tile([P, NPAIR, 2, CW], fp8, name="w_sb")

        # Spread input DMAs over the HWDGE-capable queues (SP + Act) plus
        # GpSimd SWDGE for the late chunks; issue in consumption order.
        nc.sync.dma_start(xnq_sb[:, :], xnq[:, :])
        nc.scalar.dma_start(xnt_sb[:, 0:KC], xnt[0, :, :])
        nc.sync.dma_start(w_sb[:, 0:PW, :, :], wcb[0, :, :, :, :])
        nc.scalar.dma_start(xnt_sb[:, KC:2 * KC], xnt[1, :, :])
        nc.sync.dma_start(xnt_sb[:, 2 * KC:3 * KC], xnt[2, :, :])
        nc.scalar.dma_start(w_sb[:, PW:2 * PW, :, :], wcb[1, :, :, :, :])
        nc.sync.dma_start(xnt_sb[:, 3 * KC:4 * KC], xnt[3, :, :])
        nc.gpsimd.dma_start(xnt_sb[:, 4 * KC:5 * KC], xnt[4, :, :])
        nc.sync.dma_start(w_sb[:, 2 * PW:3 * PW, :, :], wcb[2, :, :, :, :])
        nc.gpsimd.dma_start(xnt_sb[:, 5 * KC:6 * KC], xnt[5, :, :])

        # Asymmetric q-chunks: A = cols 0:512 (pair-batched, bank-aligned
        # 2KB sim slots), B = cols 512:768 (quad-batched, 1KB slots). Each
        # accumulator has an exclusive PSUM bank; psim slots are 4KB either
        # way, so one tag double-buffers both shapes. 36 exp/square instrs
        # instead of 48.
        QA, QB = 512, 256
        accA_b = pacc.tile([NCB, QA], f32, tag="accAb", name="accA_b")
        accA_t = pacc.tile([NT, QA], f32, tag="accAt", name="accA_t")
        accB_b = pacc.tile([NCB, QB], f32, tag="accBb", name="accB_b")
        accB_t = pacc.tile([NT, QB], f32, tag="accBt", name="accB_t")

        for quad in range(NPAIR // 2):
            for h in range(2):
                pr = 2 * quad + h
                ps = psim.tile([P, 2, QA], f32, tag="sim", name="psA")
                for s in range(2):
                    k = 2 * pr + s
                    nc.tensor.matmul(
                        ps[:, s, :],
                        xnt_sb[:, k * P:(k + 1) * P],
                        xnq_sb[:, 0:QA],
                        start=True,
                        stop=True,
                    )
                sb = work.tile([P, 2, QA], fp8, tag="sb", name="sbA")
                nc.scalar.activation(sb[:, :, :], ps[:, :, :], EXP,
                                     scale=scale_b)
                st = work.tile([P, 2, QA], fp8, tag="st", name="stA")
                if square_mode:
                    nc.vector.tensor_mul(st[:, :, :], sb[:, :, :],
                                         sb[:, :, :])
                else:
                    nc.scalar.activation(st[:, :, :], ps[:, :, :], EXP,
                                         scale=scale_t)
                nc.tensor.matmul(
                    accA_b[:],
                    w_sb[:, pr, :, BOFF:BOFF + NCB],
                    sb[:, :, :],
                    start=(pr == 0),
                    stop=(pr == NPAIR - 1),
                    perf_mode=DR,
                )
                nc.tensor.matmul(
                    accA_t[:],
                    w_sb[:, pr, :, TOFF:TOFF + NT],
                    st[:, :, :],
                    start=(pr == 0),
                    stop=(pr == NPAIR - 1),
                    perf_mode=DR,
                )
            psq = psim.tile([P, 4, QB], f32, tag="sim", name="psB")
            for s in range(4):
                k = 4 * quad + s
                nc.tensor.matmul(
                    psq[:, s, :],
                    xnt_sb[:, k * P:(k + 1) * P],
                    xnq_sb[:, QA:R],
                    start=True,
                    stop=True,
                )
            sbq = work.tile([P, 4, QB], fp8, tag="sb", name="sbB")
            nc.scalar.activation(sbq[:, :, :], psq[:, :, :], EXP,
                                 scale=scale_b)
            stq = work.tile([P, 4, QB], fp8, tag="st", name="stB")
            if square_mode:
                nc.vector.tensor_mul(stq[:, :, :], sbq[:, :, :], sbq[:, :, :])
            else:
                nc.scalar.activation(stq[:, :, :], psq[:, :, :], EXP,
                                     scale=scale_t)
            for h in range(2):
                pr = 2 * quad + h
                ssl = slice(2 * h, 2 * h + 2)
                nc.tensor.matmul(
                    accB_b[:],
                    w_sb[:, pr, :, BOFF:BOFF + NCB],
                    sbq[:, ssl, :],
                    start=(pr == 0),
                    stop=(pr == NPAIR - 1),
                    perf_mode=DR,
                )
                nc.tensor.matmul(
                    accB_t[:],
                    w_sb[:, pr, :, TOFF:TOFF + NT],
                    stq[:, ssl, :],
                    start=(pr == 0),
                    stop=(pr == NPAIR - 1),
                    perf_mode=DR,
                )

        stg_b = const.tile([NCB, R], f32, name="stg_b")
        stg_t = const.tile([NT, R], f32, name="stg_t")
        nc.scalar.copy(stg_b[:, 0:QA], accA_b[:])
        nc.vector.tensor_copy(stg_t[:, 0:QA], accA_t[:])
        nc.scalar.copy(stg_b[:, QA:R], accB_b[:])
        nc.vector.tensor_copy(stg_t[:, QA:R], accB_t[:])
        nc.sync.dma_start(out_b[:, :], stg_b[:, :])
        nc.sync.dma_start(out_t[:, :], stg_t[:, :])

    nc.compile()
    return nc


def _get_compiled(scale_t: float, scale_b: float, square_mode: bool):
    key = (round(scale_t, 9), round(scale_b, 9), square_mode)
    if key not in _compile_cache:
        _compile_cache[key] = _build(scale_t, scale_b, square_mode)
    return _compile_cache[key]


def _bf16(v):
    import ml_dtypes
    return np.asarray(v, np.float32).astype(ml_dtypes.bfloat16)


def _fp8(v):
    import ml_dtypes
    return np.asarray(v, np.float32).astype(ml_dtypes.float8_e4m3fn)


def _host_prep(input, temperature, targets, batch0):
    x = np.asarray(input, dtype=np.float32)
    t = float(np.clip(np.float32(temperature), MIN_T, MAX_T))
    scale_t = 1.0 / t
    scale_b = 1.0 / TEMP_BATCH
    square_mode = abs(scale_t - 2.0 * scale_b) < 1e-6

    norms = np.sqrt((x * x).sum(axis=1, keepdims=True, dtype=np.float32))
    norms = np.maximum(norms, np.float32(EPS)).astype(np.float32)
    xnb = _bf16(x / norms)                                 # device operand
    xnf = xnb.astype(np.float32)
    s_ii = (xnf * xnf).sum(axis=1, dtype=np.float32)       # device matmul diag

    # [XCH, P(d), KC]: chunk-major transposed blocks, contiguous per chunk
    xnt_t = np.ascontiguousarray(xnb.T)                    # [128, 6144]
    KC = KT // XCH * P
    xnt_in = np.ascontiguousarray(
        xnt_t.reshape(P, XCH, KC).transpose(1, 0, 2))

    tg = np.asarray(targets).astype(np.int64)
    bt = np.asarray(batch0).astype(np.int64)
    combo = tg * NB + bt

    # One-hot weights [P, NPAIR, 2, CW] -> chunk-major [WCH, P, PW, 2, CW].
    # Key j = (2*pr + s)*128 + p.
    w4 = np.zeros((P, NPAIR, 2, CW), dtype=np.float32)
    j = np.arange(N)
    p_idx = j % P
    k_idx = j // P
    pr_idx = k_idx // 2
    s_idx = k_idx % 2
    w4[p_idx, pr_idx, s_idx, BOFF + combo] = 1.0
    w4[p_idx, pr_idx, s_idx, TOFF + tg] = 1.0
    PW = NPAIR // WCH
    wcb_in = np.ascontiguousarray(
        _fp8(w4).reshape(P, WCH, PW, 2, CW).transpose(1, 0, 2, 3, 4))

    return (xnt_in, xnt_t, s_ii, tg, bt, combo, wcb_in,
            scale_t, scale_b, square_mode)


def _epilogue(acc_b, acc_t, s_ii, tg, bt, combo, weight_target, weight_batch0,
              scale_t, scale_b, square_mode):
    """acc_b: [100, N] combo sums of Sb; acc_t: [20, N] class sums of St.
    Everything here is O(N)."""
    f = np.float64
    idx = np.arange(N)
    combosum_b = acc_b.astype(f)
    classsum_t = acc_t.astype(f)
    rowsum_t = classsum_t.sum(axis=0)
    classsum_b = combosum_b.reshape(NT, NB, N).sum(axis=1)  # [20, N]

    # Diagonal values as the device computed them (fp8-rounded exp).
    sb_d = _fp8(np.exp(scale_b * s_ii.astype(np.float64))).astype(f)
    if square_mode:
        st_d = _fp8(sb_d.astype(np.float32) ** 2).astype(f)
    else:
        st_d = _fp8(np.exp(scale_t * s_ii.astype(np.float64))).astype(f)
    diag_t, diag_b = st_d, sb_d

    cnt_t = np.bincount(tg, minlength=NT)
    n_tb = np.zeros((NT, NB), dtype=np.int64)
    np.add.at(n_tb, (tg, bt), 1)

    # ---- target SNN loss ----
    own_t = classsum_t[tg, idx]
    pos_t = own_t - diag_t
    neg_t = rowsum_t - own_t
    cnt_pos = cnt_t[tg]
    cnt_neg = N - cnt_pos
    valid_t = (cnt_pos >= 2) & (cnt_neg >= 1)
    pos_s = np.where(valid_t, pos_t, 1.0)
    neg_s = np.where(valid_t, neg_t, 1.0)
    loss_i = -np.log(pos_s / (pos_s + neg_s))
    lsum = np.bincount(tg, weights=np.where(valid_t, loss_i, 0.0), minlength=NT)
    vcnt = np.bincount(tg, weights=valid_t.astype(f), minlength=NT)
    mean = lsum / np.maximum(vcnt, 1.0)
    wt_w = np.asarray(weight_target).astype(f)
    loss_target = np.where(vcnt > 0, mean * wt_w, 0.0).sum()

    # ---- batch-effect loss ----
    own_tb = combosum_b[combo, idx]
    samet = classsum_b[tg, idx]
    pos_b = own_tb - diag_b
    neg_b = samet - own_tb
    cnt_pos_b = n_tb[tg, bt]
    cnt_neg_b = cnt_t[tg] - cnt_pos_b
    valid_b = (cnt_pos_b >= 2) & (cnt_neg_b >= 1)
    pos_bs = np.where(valid_b, pos_b, 1.0)
    neg_bs = np.where(valid_b, neg_b, 1.0)
    loss_bi = -np.log(pos_bs / (pos_bs + neg_bs))
    inv = np.where(valid_b, 1.0 / np.where(valid_b, loss_bi, 1.0), 0.0)
    lsum_b = np.bincount(bt, weights=inv, minlength=NB)
    vcnt_b = np.bincount(bt, weights=valid_b.astype(f), minlength=NB)
    mean_b = lsum_b / np.maximum(vcnt_b, 1.0)
    wb_w = np.asarray(weight_batch0).astype(f)
    loss_batch = np.where(vcnt_b > 0, mean_b * wb_w, 0.0).sum()

    return np.float32(0.9 * loss_target + 0.1 * loss_batch)


def _run_with_retry(nc, in_maps, core_ids, attempts=3):
    import time as _time

    from concourse.bass_utils import run_bass_kernel_spmd

    for i in range(attempts):
        try:
            return run_bass_kernel_spmd(nc, in_maps, core_ids)
        except Exception:
            if i == attempts - 1:
                raise
            _time.sleep(90)  # transient NRT device errors clear after a pause


def kernel(input, temperature, weight_target, weight_batch0, targets, batch0):
    global LAST_RESULT

    (xnt_in, xnt_t, s_ii, tg, bt, combo, wcb_in,
     scale_t, scale_b, square_mode) = _host_prep(input, temperature,
                                                 targets, batch0)

    nc = _get_compiled(scale_t, scale_b, square_mode)

    in_maps = [
        {
            "xnt": xnt_in,
            "xnq": np.ascontiguousarray(xnt_t[:, c * R:(c + 1) * R]),
            "wcb": wcb_in,
        }
        for c in range(NCORES)
    ]
    LAST_RESULT = _run_with_retry(nc, in_maps, list(range(NCORES)))
    acc_b = np.concatenate(
        [LAST_RESULT.results[c]["out_b"] for c in range(NCORES)], axis=1)
    acc_t = np.concatenate(
        [LAST_RESULT.results[c]["out_t"] for c in range(NCORES)], axis=1)

    return _epilogue(acc_b, acc_t, s_ii, tg, bt, combo, weight_target,
                     weight_batch0, scale_t, scale_b, square_mode)


# revision 28
# speedup vs baseline: 1.0032x; 1.0032x over previous
"""Trainium2 Bass kernel for CustomSNNLoss (nn_CustomSNNLoss_36429912604816).

Strategy (data-parallel over rows of the NxN similarity):
  - Host: normalize x, quantize to bf16, pre-transpose to xnT [128, N];
    build per-key one-hot class matrices in fp8 (exact 0/1 values).
  - Each of the 8 cores owns R = 768 query rows, split into asymmetric
    q-chunks A (512 cols, pair-batched 2KB-aligned sim slots) and B
    (256 cols, quad-batched 1KB sim slots) so exp/square run as 36 wide
    instructions with PSUM = 4 sim banks + 4 exclusive acc banks:
        sim      = xnT[:, ktile].T @ xnq          (bf16 matmul -> PSUM f32)
        Sb       = exp(2 * sim) batch-batched     (ScalarE, fp8 out)
        St       = Sb * Sb                        (DVE, fp8)
        acc_b   += onehot_combo[pair].T  @ Sb     (fp8 DoubleRow, 256-key
        acc_t   += onehot_target[pair].T @ St      contraction per matmul)
    fp8 quantization of the exp values is harmless: errors average out in
    the ~300-element segment sums and biases cancel in pos/(pos+neg)
    (host-emulated end-to-end rel err ~1e-6).
  - Host epilogue (O(N)): per-row pos/neg sums from the class sums, -log
    losses, validity masks, class-weighted means, final scalar. Diagonal
    subtraction uses host emulation of the device's bf16/fp8 rounding.
"""

import numpy as np

N, D = 6144, 128
P = 128                 # partitions / contraction tile
NCORES = 8
R = N // NCORES         # 768 query rows per core
KT = N // P             # 48 key tiles
NPAIR = KT // 2         # 24 key-tile pairs (DoubleRow contracts 2 tiles)
QC = 2                  # query chunks per core
QF = R // QC            # 384 free-dim per matmul
NT, NB = 20, 5          # target classes, batch keys
NCB = NT * NB           # 100 combined classes
# DoubleRow LDWEIGHTS requires the k-subtile stride %16==0 and 16-aligned
# slice offsets: pad the one-hot row to 144 cols; target at [0:20],
# combo at [32:132].
CW = 144
TOFF, BOFF = 0, 32
XCH = 6                 # xnt DMA chunks (8 k-tiles each)
WCH = 3                 # weight DMA chunks (8 pairs each)
MIN_T, MAX_T = 0.1, 1.0
TEMP_BATCH = 0.5
EPS = 1e-8

_compile_cache = {}
LAST_RESULT = None  # BassKernelResults from the most recent device run


def _build(scale_t: float, scale_b: float, square_mode: bool):
    from contextlib import ExitStack

    import concourse.bacc as bacc
    import concourse.mybir as mybir
    import concourse.tile as tile

    f32 = mybir.dt.float32
    bf16 = mybir.dt.bfloat16
    fp8 = mybir.dt.float8e4
    EXP = mybir.ActivationFunctionType.Exp
    DR = mybir.MatmulPerfMode.DoubleRow

    nc = bacc.Bacc("TRN2", target_bir_lowering=False, debug=False,
                   enable_asserts=False)

    KC = KT // XCH * P      # key columns per xnt chunk
    xnt = nc.dram_tensor("xnt", [XCH, P, KC], bf16, kind="ExternalInput").ap()
    xnq = nc.dram_tensor("xnq", [P, R], bf16, kind="ExternalInput").ap()
    PW = NPAIR // WCH       # pairs per weight chunk (8)
    wcb = nc.dram_tensor("wcb", [WCH, P, PW, 2, CW], fp8,
                         kind="ExternalInput").ap()
    out_b = nc.dram_tensor("out_b", [NCB, R], f32, kind="ExternalOutput").ap()
    out_t = nc.dram_tensor("out_t", [NT, R], f32, kind="ExternalOutput").ap()

    with tile.TileContext(nc) as tc, ExitStack() as ctx:
        const = ctx.enter_context(tc.tile_pool(name="const", bufs=1))
        work = ctx.enter_context(tc.tile_pool(name="work", bufs=4))
        psim = ctx.enter_context(tc.tile_pool(name="psim", bufs=2, space="PSUM"))
        pacc = ctx.enter_context(tc.tile_pool(name="pacc", bufs=1, space="PSUM"))

        xnt_sb = const.tile([P, N], bf16, name="xnt_sb")
        xnq_sb = const.tile([P, R], bf16, name="xnq_sb")
        w_sb = const.tile([P, NPAIR, 2, CW], fp8, name="w_sb")

        # Spread input DMAs over the HWDGE-capable queues (SP + Act) plus
        # GpSimd SWDGE for the late chunks; issue in consumption order.
        nc.sync.dma_start(xnq_sb[:, :], xnq[:, :])
        nc.scalar.dma_start(xnt_sb[:, 0:KC], xnt[0, :, :])
        nc.sync.dma_start(w_sb[:, 0:PW, :, :], wcb[0, :, :, :, :])
        nc.scalar.dma_start(xnt_sb[:, KC:2 * KC], xnt[1, :, :])
        nc.sync.dma_start(xnt_sb[:, 2 * KC:3 * KC], xnt[2, :, :])
        nc.scalar.dma_start(w_sb[:, PW:2 * PW, :, :], wcb[1, :, :, :, :])
        nc.sync.dma_start(xnt_sb[:, 3 * KC:4 * KC], xnt[3, :, :])
        nc.scalar.dma_start(xnt_sb[:, 4 * KC:5 * KC], xnt[4, :, :])
        nc.sync.dma_start(w_sb[:, 2 * PW:3 * PW, :, :], wcb[2, :, :, :, :])
        nc.sync.dma_start(xnt_sb[:, 5 * KC:6 * KC], xnt[5, :, :])

        # Asymmetric q-chunks: A = cols 0:512 (pair-batched, bank-aligned
        # 2KB sim slots), B = cols 512:768 (quad-batched, 1KB slots). Each
        # accumulator has an exclusive PSUM bank; psim slots are 4KB either
        # way, so one tag double-buffers both shapes. 36 exp/square instrs
        # instead of 48.
        QA, QB = 512, 256
        accA_b = pacc.tile([NCB, QA], f32, tag="accAb", name="accA_b")
        accA_t = pacc.tile([NT, QA], f32, tag="accAt", name="accA_t")
        accB_b = pacc.tile([NCB, QB], f32, tag="accBb", name="accB_b")
        accB_t = pacc.tile([NT, QB], f32, tag="accBt", name="accB_t")

        for quad in range(NPAIR // 2):
            for h in range(2):
                pr = 2 * quad + h
                ps = psim.tile([P, 2, QA], f32, tag="sim", name="psA")
                for s in range(2):
                    k = 2 * pr + s
                    nc.tensor.matmul(
                        ps[:, s, :],
                        xnt_sb[:, k * P:(k + 1) * P],
                        xnq_sb[:, 0:QA],
                        start=True,
                        stop=True,
                    )
                sb = work.tile([P, 2, QA], fp8, tag="sb", name="sbA")
                nc.scalar.activation(sb[:, :, :], ps[:, :, :], EXP,
                                     scale=scale_b)
                st = work.tile([P, 2, QA], fp8, tag="st", name="stA")
                if square_mode:
                    nc.vector.tensor_mul(st[:, :, :], sb[:, :, :],
                                         sb[:, :, :])
                else:
                    nc.scalar.activation(st[:, :, :], ps[:, :, :], EXP,
                                         scale=scale_t)
                nc.tensor.matmul(
                    accA_b[:],
                    w_sb[:, pr, :, BOFF:BOFF + NCB],
                    sb[:, :, :],
                    start=(pr == 0),
                    stop=(pr == NPAIR - 1),
                    perf_mode=DR,
                )
                nc.tensor.matmul(
                    accA_t[:],
                    w_sb[:, pr, :, TOFF:TOFF + NT],
                    st[:, :, :],
                    start=(pr == 0),
                    stop=(pr == NPAIR - 1),
                    perf_mode=DR,
                )
            psq = psim.tile([P, 4, QB], f32, tag="sim", name="psB")
            for s in range(4):
                k = 4 * quad + s
                nc.tensor.matmul(
                    psq[:, s, :],
                    xnt_sb[:, k * P:(k + 1) * P],
                    xnq_sb[:, QA:R],
                    start=True,
                    stop=True,
                )
            sbq = work.tile([P, 4, QB], fp8, tag="sb", name="sbB")
            nc.scalar.activation(sbq[:, :, :], psq[:, :, :], EXP,
                                 scale=scale_b)
            stq = work.tile([P, 4, QB], fp8, tag="st", name="stB")
            if square_mode:
                nc.vector.tensor_mul(stq[:, :, :], sbq[:, :, :], sbq[:, :, :])
            else:
                nc.scalar.activation(stq[:, :, :], psq[:, :, :], EXP,
                                     scale=scale_t)
            for h in range(2):
                pr = 2 * quad + h
                ssl = slice(2 * h, 2 * h + 2)
                nc.tensor.matmul(
                    accB_b[:],
                    w_sb[:, pr, :, BOFF:BOFF + NCB],
                    sbq[:, ssl, :],
                    start=(pr == 0),
                    stop=(pr == NPAIR - 1),
                    perf_mode=DR,
                )
                nc.tensor.matmul(
                    accB_t[:],
                    w_sb[:, pr, :, TOFF:TOFF + NT],
                    stq[:, ssl, :],
                    start=(pr == 0),
                    stop=(pr == NPAIR - 1),
                    perf_mode=DR,
                )

        stg_b = const.tile([NCB, R], f32, name="stg_b")
        stg_t = const.tile([NT, R], f32, name="stg_t")
        nc.scalar.copy(stg_b[:, 0:QA], accA_b[:])
        nc.vector.tensor_copy(stg_t[:, 0:QA], accA_t[:])
        nc.scalar.copy(stg_b[:, QA:R], accB_b[:])
        nc.vector.tensor_copy(stg_t[:, QA:R], accB_t[:])
        nc.sync.dma_start(out_b[:, :], stg_b[:, :])
        nc.sync.dma_start(out_t[:, :], stg_t[:, :])

    nc.compile()
    return nc


def _get_compiled(scale_t: float, scale_b: float, square_mode: bool):
    key = (round(scale_t, 9), round(scale_b, 9), square_mode)
    if key not in _compile_cache:
        _compile_cache[key] = _build(scale_t, scale_b, square_mode)
    return _compile_cache[key]


def _bf16(v):
    import ml_dtypes
    return np.asarray(v, np.float32).astype(ml_dtypes.bfloat16)


def _fp8(v):
    import ml_dtypes
    return np.asarray(v, np.float32).astype(ml_dtypes.float8_e4m3fn)


def _host_prep(input, temperature, targets, batch0):
    x = np.asarray(input, dtype=np.float32)
    t = float(np.clip(np.float32(temperature), MIN_T, MAX_T))
    scale_t = 1.0 / t
    scale_b = 1.0 / TEMP_BATCH
    square_mode = abs(scale_t - 2.0 * scale_b) < 1e-6

    norms = np.sqrt((x * x).sum(axis=1, keepdims=True, dtype=np.float32))
    norms = np.maximum(norms, np.float32(EPS)).astype(np.float32)
    xnb = _bf16(x / norms)                                 # device operand
    xnf = xnb.astype(np.float32)
    s_ii = (xnf * xnf).sum(axis=1, dtype=np.float32)       # device matmul diag

    # [XCH, P(d), KC]: chunk-major transposed blocks, contiguous per chunk
    xnt_t = np.ascontiguousarray(xnb.T)                    # [128, 6144]
    KC = KT // XCH * P
    xnt_in = np.ascontiguousarray(
        xnt_t.reshape(P, XCH, KC).transpose(1, 0, 2))

    tg = np.asarray(targets).astype(np.int64)
    bt = np.asarray(batch0).astype(np.int64)
    combo = tg * NB + bt

    # One-hot weights [P, NPAIR, 2, CW] -> chunk-major [WCH, P, PW, 2, CW].
    # Key j = (2*pr + s)*128 + p.
    w4 = np.zeros((P, NPAIR, 2, CW), dtype=np.float32)
    j = np.arange(N)
    p_idx = j % P
    k_idx = j // P
    pr_idx = k_idx // 2
    s_idx = k_idx % 2
    w4[p_idx, pr_idx, s_idx, BOFF + combo] = 1.0
    w4[p_idx, pr_idx, s_idx, TOFF + tg] = 1.0
    PW = NPAIR // WCH
    wcb_in = np.ascontiguousarray(
        _fp8(w4).reshape(P, WCH, PW, 2, CW).transpose(1, 0, 2, 3, 4))

    return (xnt_in, xnt_t, s_ii, tg, bt, combo, wcb_in,
            scale_t, scale_b, square_mode)


def _epilogue(acc_b, acc_t, s_ii, tg, bt, combo, weight_target, weight_batch0,
              scale_t, scale_b, square_mode):
    """acc_b: [100, N] combo sums of Sb; acc_t: [20, N] class sums of St.
    Everything here is O(N)."""
    f = np.float64
    idx = np.arange(N)
    combosum_b = acc_b.astype(f)
    classsum_t = acc_t.astype(f)
    rowsum_t = classsum_t.sum(axis=0)
    classsum_b = combosum_b.reshape(NT, NB, N).sum(axis=1)  # [20, N]

    # Diagonal values as the device computed them (fp8-rounded exp).
    sb_d = _fp8(np.exp(scale_b * s_ii.astype(np.float64))).astype(f)
    if square_mode:
        st_d = _fp8(sb_d.astype(np.float32) ** 2).astype(f)
    else:
        st_d = _fp8(np.exp(scale_t * s_ii.astype(np.float64))).astype(f)
    diag_t, diag_b = st_d, sb_d

    cnt_t = np.bincount(tg, minlength=NT)
    n_tb = np.zeros((NT, NB), dtype=np.int64)
    np.add.at(n_tb, (tg, bt), 1)

    # ---- target SNN loss ----
    own_t = classsum_t[tg, idx]
    pos_t = own_t - diag_t
    neg_t = rowsum_t - own_t
    cnt_pos = cnt_t[tg]
    cnt_neg = N - cnt_pos
    valid_t = (cnt_pos >= 2) & (cnt_neg >= 1)
    pos_s = np.where(valid_t, pos_t, 1.0)
    neg_s = np.where(valid_t, neg_t, 1.0)
    loss_i = -np.log(pos_s / (pos_s + neg_s))
    lsum = np.bincount(tg, weights=np.where(valid_t, loss_i, 0.0), minlength=NT)
    vcnt = np.bincount(tg, weights=valid_t.astype(f), minlength=NT)
    mean = lsum / np.maximum(vcnt, 1.0)
    wt_w = np.asarray(weight_target).astype(f)
    loss_target = np.where(vcnt > 0, mean * wt_w, 0.0).sum()

    # ---- batch-effect loss ----
    own_tb = combosum_b[combo, idx]
    samet = classsum_b[tg, idx]
    pos_b = own_tb - diag_b
    neg_b = samet - own_tb
    cnt_pos_b = n_tb[tg, bt]
    cnt_neg_b = cnt_t[tg] - cnt_pos_b
    valid_b = (cnt_pos_b >= 2) & (cnt_neg_b >= 1)
    pos_bs = np.where(valid_b, pos_b, 1.0)
    neg_bs = np.where(valid_b, neg_b, 1.0)
    loss_bi = -np.log(pos_bs / (pos_bs + neg_bs))
    inv = np.where(valid_b, 1.0 / np.where(valid_b, loss_bi, 1.0), 0.0)
    lsum_b = np.bincount(bt, weights=inv, minlength=NB)
    vcnt_b = np.bincount(bt, weights=valid_b.astype(f), minlength=NB)
    mean_b = lsum_b / np.maximum(vcnt_b, 1.0)
    wb_w = np.asarray(weight_batch0).astype(f)
    loss_batch = np.where(vcnt_b > 0, mean_b * wb_w, 0.0).sum()

    return np.float32(0.9 * loss_target + 0.1 * loss_batch)


def _run_with_retry(nc, in_maps, core_ids, attempts=3):
    import time as _time

    from concourse.bass_utils import run_bass_kernel_spmd

    for i in range(attempts):
        try:
            return run_bass_kernel_spmd(nc, in_maps, core_ids)
        except Exception:
            if i == attempts - 1:
                raise
            _time.sleep(90)  # transient NRT device errors clear after a pause


def kernel(input, temperature, weight_target, weight_batch0, targets, batch0):
    global LAST_RESULT

    (xnt_in, xnt_t, s_ii, tg, bt, combo, wcb_in,
     scale_t, scale_b, square_mode) = _host_prep(input, temperature,
                                                 targets, batch0)

    nc = _get_compiled(scale_t, scale_b, square_mode)

    in_maps = [
        {
            "xnt": xnt_in,
            "xnq": np.ascontiguousarray(xnt_t[:, c * R:(c + 1) * R]),
            "wcb": wcb_in,
        }
        for c in range(NCORES)
    ]
    LAST_RESULT = _run_with_retry(nc, in_maps, list(range(NCORES)))
    acc_b = np.concatenate(
        [LAST_RESULT.results[c]["out_b"] for c in range(NCORES)], axis=1)
    acc_t = np.concatenate(
        [LAST_RESULT.results[c]["out_t"] for c in range(NCORES)], axis=1)

    return _epilogue(acc_b, acc_t, s_ii, tg, bt, combo, weight_target,
                     weight_batch0, scale_t, scale_b, square_mode)
